# revision 20
# baseline (speedup 1.0000x reference)
"""Trainium2 distributed kernel for a linear-recurrence associative scan.

    h_t = g_t * h_{t-1} + x_t  along the sequence axis (N=8192)

Shapes: gates/inputs [B=4, N=8192, D=1024] f32.

Strategy: the scan is independent per (b, d) lane -> 4096 lanes of length
8192.  Shard lanes across the 8 NeuronCores (512 lanes each), laid out
lane-major so each SBUF partition holds one lane's contiguous sequence and
the hardware scan instruction (tensor_tensor_scan: state = g*state + x along
the free dim, one recurrence per partition) does the whole recurrence at
vector-engine line rate.  No collectives needed.

Gates and inputs are packed into one [512, 2*N] array per core so each
lane-tile needs a single input DMA (8 DMAs total -> every DMA gets its own
completion-sem lane).  The walrus backend allows only ONE sync-wait per
instruction, so multi-dependency points are preceded by tiny same-engine
"absorber" ops that each carry one wait; later instructions then rely on the
engine's observed vector clock instead of their own waits.
"""

import numpy as np

B, N, D = 4, 8192, 1024
N_CORES = 8
LANES = B * D  # 4096 independent recurrences
LANES_PER_CORE = LANES // N_CORES  # 512
P = 128  # SBUF partitions
LANE_TILES = LANES_PER_CORE // P  # 4

_NC_CACHE = None


def _build_bass():
    import concourse.bass as bass
    import concourse.tile as tile
    from concourse import mybir
    from concourse.vector_clock import ScopedClock, VectorClock

    class OneWaitDrainTC(tile.TileContext):
        """This walrus/ISA generation encodes at most ONE sync-wait per
        instruction, but Tile's kernel-tail drain waits on every live
        semaphore at once.  Split those waits into a ladder of single-wait
        NOPs on the drain's queue first; the drain's own waits then elide
        against the queue's observed clock."""

        def _drain_and_barrier(self, tick_clock, wait_clock):
            full = tick_clock.global_clock
            n = len(full)
            for proc in range(n):
                t = full[proc]
                if t <= 0:
                    continue
                partial = VectorClock([0] * n)
                partial.require_at_least(proc, t)
                nop = self.nc.sync.nop(hint=f"drainwait{proc}")
                wait_clock.add_sem_waits(nop.ins, ScopedClock({None: partial}))
            # replicate super()._drain_and_barrier but leave the drain
            # wait-free: the nop ladder above already enforced every sem.
            self.nc.sync.drain()
            self.nc.all_engine_barrier()
            assert self.sems is not None
            popped = self.nc._tile_sem_poison_stack.pop()
            assert popped is self._sem_poison
            self.nc.clear_and_free_semaphores(list(self.sems.allocated().values()))
            self.nc.all_engine_barrier()

    f32 = mybir.dt.float32
    nc = bass.Bass()
    gx_ext = nc.declare_dram_parameter(
        "gx", [LANES_PER_CORE, 2 * N], f32, isOutput=False
    )
    o_ext = nc.declare_dram_parameter("out", [LANES_PER_CORE, N], f32, isOutput=True)

    with OneWaitDrainTC(nc) as tc:
        with (
            tc.tile_pool(name="gx", bufs=2) as gxp,
            tc.tile_pool(name="o", bufs=2) as op,
            tc.tile_pool(name="vscr", bufs=1) as vscrp,
        ):
            # DVE-private scratch; one-time init so later reads depend only
            # on an ancient DVE tick (no fresh same-engine RAW wait).
            dve_scr = vscrp.tile([P, 32], f32)
            nc.vector.memset(dve_scr[:], 0.0)
            gx_tiles, o_tiles, in_dmas = [], [], []

            def gx_dram(lt):
                rows = slice(lt * P, (lt + 1) * P)
                return gx_ext[rows, :].rearrange("p (a n) -> p a n", n=N)

            def issue_in(lt):
                # All DMAs ride the SP (HWDGE) queue.  in(lt>=2)'s WAR on
                # scan(lt-2) is subsumed by out(lt-2)'s DVE wait earlier on
                # the same queue; its slot-WAW lane wait (if any) is its one
                # permitted sync-wait.
                t = gxp.tile([P, 2, N], f32)
                d = nc.sync.dma_start(out=t[:], in_=gx_dram(lt))
                gx_tiles.append(t)
                in_dmas.append(d)

            def do_scan(lt):
                gxt = gx_tiles[lt]
                ot = op.tile([P, N], f32)
                o_tiles.append(ot)
                # absorber: carries the input-DMA completion wait for DVE
                # (unique dest cell per lt -> no same-engine WAW waits)
                nc.vector.tensor_copy(dve_scr[:, 2 + lt : 3 + lt], gxt[:, 0, 0:1])
                # absorber: reads the previous scan's output so the DVE
                # queue observes a tick >= scan(lt-1); that subsumes the
                # scan's same-engine WAW on its recycled output slot.
                src = dve_scr[:, 1:2] if lt == 0 else o_tiles[lt - 1][:, 0:1]
                nc.vector.tensor_copy(dve_scr[:, 10 + lt : 11 + lt], src)
                # absorber: its write to ot carries the WAR wait on
                # out-DMA(lt-2) for slot reuse
                nc.vector.tensor_copy(ot[:, 0:1], dve_scr[:, 1:2])
                nc.vector.tensor_tensor_scan(
                    ot[:],
                    gxt[:, 0, :],
                    gxt[:, 1, :],
                    0.0,
                    mybir.AluOpType.mult,
                    mybir.AluOpType.add,
                )

            def issue_out(lt):
                rows = slice(lt * P, (lt + 1) * P)
                nc.sync.dma_start(out=o_ext[rows, :], in_=o_tiles[lt][:])

            # Software-pipelined order; on the SP queue out(lt-2) must
            # precede in(lt) (WAR subsumption), and in(lt) must precede
            # out(lt-1) so input streaming is never blocked by an unmet
            # scan wait.
            issue_in(0)
            issue_in(1)
            # Serialize in(1) behind in(0): otherwise the two startup DMAs
            # share HBM bandwidth and finish together, delaying scan(0) (and
            # the whole downstream pipe) by a full scan-length bubble.
            tile.add_dep_helper(
                in_dmas[1].ins, in_dmas[0].ins, sync=True, reason="startup order"
            )
            do_scan(0)
            issue_out(0)
            do_scan(1)
            issue_in(2)
            # Chain every input behind its predecessor: inputs never share
            # HBM bandwidth with each other, so each scan starts right at
            # its own input's completion and consecutive scans never
            # contend on the vector engine.  Each dep wait also puts the
            # predecessor's completion lane into the SP queue's observed
            # clock, which is what elides the downstream WAW and
            # lane-reuse waits (1-wait legality).
            tile.add_dep_helper(
                in_dmas[2].ins, in_dmas[1].ins, sync=True, reason="input chain"
            )
            issue_out(1)
            do_scan(2)

            # Tile 3 arrives as two chained seq-pieces (the second deferred
            # behind the first) and leaves as two output pieces, shrinking
            # the final serial chain in3 -> scan3 -> out3.  These are DMAs
            # 9 and 10; their recycled completion lanes (0 and 1) were
            # already observed by the SP queue via the input-chain deps, so
            # each out-piece carries only its scan wait.  The split point
            # minimizes f*stream + max((1-f)*stream, f*scan) +
            # (1-f)*(scan + out): with stream ~2.6x scan, f ~= 0.72.
            H = 5888
            t3 = gxp.tile([P, 2, N], f32, tag="t")
            gx_tiles.append(t3)
            d3a = nc.sync.dma_start(out=t3[:, :, 0:H], in_=gx_dram(3)[:, :, 0:H])
            d3b = nc.sync.dma_start(out=t3[:, :, H:N], in_=gx_dram(3)[:, :, H:N])
            tile.add_dep_helper(
                d3a.ins, in_dmas[2].ins, sync=True, reason="input chain"
            )
            tile.add_dep_helper(
                d3b.ins, d3a.ins, sync=True, reason="tail halves order"
            )
            issue_out(2)

            ot3 = op.tile([P, N], f32, tag="ot")
            o_tiles.append(ot3)
            nc.vector.tensor_copy(dve_scr[:, 6:7], t3[:, 0, 0:1])  # in3a lane
            nc.vector.tensor_copy(dve_scr[:, 7:8], t3[:, 0, H : H + 1])  # in3b lane
            nc.vector.tensor_copy(dve_scr[:, 14:15], o_tiles[2][:, 0:1])  # scan2 tick
            nc.vector.tensor_copy(ot3[:, 0:1], dve_scr[:, 1:2])  # WAR out1 lane
            nc.vector.tensor_tensor_scan(
                ot3[:, 0:H],
                t3[:, 0, 0:H],
                t3[:, 1, 0:H],
                0.0,
                mybir.AluOpType.mult,
                mybir.AluOpType.add,
            )
            nc.vector.tensor_tensor_scan(
                ot3[:, H:N],
                t3[:, 0, H:N],
                t3[:, 1, H:N],
                ot3[:, H - 1 : H],
                mybir.AluOpType.mult,
                mybir.AluOpType.add,
            )
            rows3 = slice(3 * P, 4 * P)
            nc.sync.dma_start(out=o_ext[rows3, 0:H], in_=ot3[:, 0:H])
            nc.sync.dma_start(out=o_ext[rows3, H:N], in_=ot3[:, H:N])
    return nc


def _get_nc():
    global _NC_CACHE
    if _NC_CACHE is None:
        _NC_CACHE = _build_bass()
    return _NC_CACHE


def kernel(gates: np.ndarray, inputs: np.ndarray) -> np.ndarray:
    import os

    # The axon client here has no NTFF profile hook (antenv.axon_hooks);
    # make sure run_bass_kernel_spmd never takes the trace path even if
    # BASS_TRACE is set in the environment.
    os.environ["BASS_NEVER_TRACE"] = "1"
    from concourse.bass_utils import run_bass_kernel_spmd

    gates = np.asarray(gates, dtype=np.float32)
    inputs = np.asarray(inputs, dtype=np.float32)

    # [B, N, D] -> lane-major [B*D, N]; pack gates|inputs along columns
    gt = np.ascontiguousarray(gates.transpose(0, 2, 1)).reshape(LANES, N)
    xt = np.ascontiguousarray(inputs.transpose(0, 2, 1)).reshape(LANES, N)
    gx = np.concatenate([gt, xt], axis=1)  # [LANES, 2N]

    in_maps = [
        {"gx": gx[i * LANES_PER_CORE : (i + 1) * LANES_PER_CORE]}
        for i in range(N_CORES)
    ]
    res = run_bass_kernel_spmd(_get_nc(), in_maps, core_ids=list(range(N_CORES)))
    out = np.concatenate([res.results[i]["out"] for i in range(N_CORES)], axis=0)
    # [B*D, N] -> [B, N, D]
    return np.ascontiguousarray(out.reshape(B, D, N).transpose(0, 2, 1))
